# revision 63
# baseline (speedup 1.0000x reference)
"""CapsNet EM-routing conv-caps kernel for 8 TRN2 NeuronCores.

Data-parallel over b*oh*ow (256 positions -> 32 per core, 2 halves of 16,
software-pipelined so h1's DVE-heavy m-step overlaps h0's ACT/PE-heavy
e-step). Big-tensor layout: partition = c*4+k (ck), free = [i, pos, n]
(n innermost) in fp16 for the DVE 2x_1p fast path.

Structure vs reference:
 - a_in folding: sum_c r == 1 always, so the m-step r-normalization is
   r*s with s = a/(a+eps); s, w0 = s/(sum_n s + 32 eps) and rsum0 come
   from host. iter-0 mu is a pure PE matmul: sum_(j,n) W * (pose*w0).
 - n-reductions (288) as fp16 pairwise-halving tree adds (DVE/Pool mix)
   + final fp32 tensor_reduce.
 - e-step: (v-mu)*rs fused per (i,pos) as one 4x TensorScalarPtr op;
   squares on ACT; qa = sum_ik on PE (maskck matmuls, PSUM accum);
   exp fused with per-pos bias (+10 shift after max-sub keeps fp16 E2
   in range); sum_c / g-broadcast / c->ck expansion on PE.
 - cost_stdv == sqrt(EPS) exactly (devsum is a sum of deviations, zero
   in exact arithmetic), so a_out collapses to one matmul + sigmoid.
"""
import sys
sys.path.insert(0, "/opt/trn_rl_repo")
import math
import numpy as np
from contextlib import ExitStack

import concourse.bass as bass
import concourse.tile as tile
from concourse import mybir
from concourse import bass_utils

F32 = mybir.dt.float32
F16 = mybir.dt.float16
AX = mybir.AxisListType
OP = mybir.AluOpType
ACT = mybir.ActivationFunctionType

B_CAPS = 32; C_CAPS = 32; KK = 3; P = 4; PSIZE = 16; ITERS = 3
EPS = 1e-8; LAM = 1e-6; LN_2PI = math.log(2.0 * math.pi)
NB = KK * KK * B_CAPS          # 288 input capsules
NPOS_CORE = 32
HALF = 16
N_CORES = 8
SHIFT = 10.0                   # global exp shift (cancels in normalization)
LN_SQRT2 = 0.5 * math.log(2.0)

_cache = {}
DBG = None          # e.g. ("rsum", 1) dumps that tile for half 0 into DBGO
CFG = dict(t2a_pool=True, t2b_pool=False, t3_pool=True, t3s_pool=False,
           sq_pool=0, t2c_pool=False, gmul_pool=False, eops_pool=0)   # sq_pool: 0=none, 1=i%2==0, 2=all



def _fix_excess_waits(nc, max_keep=1, max_per_nop=1):
    """TRN2 walrus codegen accepts very few sync-wait commands per
    instruction; move excess waits onto preceding same-engine NoOps."""
    for f in nc.m.functions:
        for blk in f.blocks:
            new_insts = []
            for inst in blk.instructions:
                si = getattr(inst, "sync_info", None)
                waits = list(si.on_wait) if si and si.on_wait else []
                if len(waits) > max_keep:
                    excess = waits[:-max_keep]
                    keep = waits[-max_keep:]
                    for ci in range(0, len(excess), max_per_nop):
                        new_insts.append(mybir.InstNoOp(
                            name=f"{inst.name}-wf{ci}", ins=[], outs=[],
                            engine=inst.engine,
                            sync_info=mybir.SyncInfo(
                                on_wait=excess[ci:ci + max_per_nop],
                                on_update=[])))
                    inst.sync_info = mybir.SyncInfo(
                        on_wait=keep, on_update=list(si.on_update or []))
                new_insts.append(inst)
            blk.instructions = new_insts


def _build_nc():
    nc = bass.Bass("TRN2", target_bir_lowering=False, debug=False,
                   num_devices=N_CORES)
    dram = {}
    def din(name, shape, dt=F32):
        dram[name] = nc.dram_tensor(name, list(shape), dt, kind="ExternalInput").ap()
    din("WP", [4, NB, 256], F16)         # [j, n, (wmat c*4+k | pose pos*4+i)]
    din("WPF", [4 * NB, 256], F16)       # flat (j,n): [wmat | pose*w0]
    din("W0REP", [128, 32, NB], F16)     # iter-0 coeff replicated over ck
    din("S16", [32, NB], F32)            # s = a/(a+eps), partition=pos
    din("RSUM0", [128, 32], F32)         # sum_n s / 32, replicated over ck
    din("BETAU", [128, 4], F32)          # beta_u[c, i*4+k] at part c*4+k, free i
    din("BETAA", [32, 1], F32)
    din("MASKCK", [128, 32], F16)        # delta(part//4 == col)
    din("MASKCK32", [128, 32], F32)
    din("REPCK", [32, 128], F16)         # delta(col//4 == row)
    din("ECOL", [32, 16, 16], F16)       # [:,p,:] = sum_c -> partition p
    din("EYER", [16, 16, 128], F16)      # [:,p,:] = bcast row p -> 128 parts
    OUT = nc.dram_tensor("OUT", [NPOS_CORE, 544], F32, kind="ExternalOutput").ap()
    DBGO = nc.dram_tensor("DBGO", [128, 4608], F32, kind="ExternalOutput").ap()
    def tap(name, it, ap):
        if DBG == (name, it):
            shp = ap.shape
            flat = ap.rearrange("p ... -> p (...)") if len(shp) > 2 else ap
            nc.sync.dma_start(out=DBGO[0:flat.shape[0], 0:flat.shape[1]], in_=flat)

    with tile.TileContext(nc) as tc, ExitStack() as ctx:
        sb = ctx.enter_context(tc.tile_pool(name="sb", bufs=1))
        tsh = ctx.enter_context(tc.tile_pool(name="tsh", bufs=1))
        dep = ctx.enter_context(tc.tile_pool(name="dep", bufs=2))

        # ---- persistent consts ----
        betau = sb.tile([128, 4], F32, tag="betau")
        betaa = sb.tile([32, 1], F32, tag="betaa")
        maskck = sb.tile([128, 32], F16, tag="maskck")
        maskck32 = sb.tile([128, 32], F32, tag="maskck32")
        repck = sb.tile([32, 128], F16, tag="repck")
        ecol = sb.tile([32, 16, 16], F16, tag="ecol")
        eyer = sb.tile([16, 16, 128], F16, tag="eyer")
        rsum0t = sb.tile([128, 32], F32, tag="rsum0t")
        nls2 = sb.tile([128, 1], F32, tag="nls2")
        nc.vector.memset(nls2[:], -LN_SQRT2)
        j32 = sb.tile([32, 32], F32, tag="j32")
        nc.vector.memset(j32[:], 1.0 / 32.0)
        balam = sb.tile([32, 1], F32, tag="balam")

        Vh, WEh, Rh, S16h = [], [], [], []
        SQt = sb.tile([128, 4, 4, NB], F16, tag="SQ", name="SQt")
        SQh = [SQt, SQt]
        for h in range(2):
            Vh.append(sb.tile([128, 4, HALF, NB], F16, tag=f"V{h}", name=f"V{h}"))
            WEh.append(sb.tile([128, HALF, NB], F16, tag=f"WE{h}", name=f"WE{h}"))
            Rh.append(sb.tile([128, HALF, NB], F16, tag=f"R{h}", name=f"Rt{h}"))
            pass
            s16 = sb.tile([16, NB], F32, tag=f"S16{h}", name=f"s16_{h}")
            S16h.append(s16)

        # per-half state carried across iterations
        st = [dict() for _ in range(2)]

        # ---- EM pools (opened after votes psum pool closes) ----
        pools = {}

        def open_em_pools():
            pools["psA"] = ctx.enter_context(tc.tile_pool(name="psA", bufs=2, space="PSUM"))
            pools["psB"] = ctx.enter_context(tc.tile_pool(name="psB", bufs=2, space="PSUM"))
        # ---- votes: v[(ck), i, pos, n] = sum_j W[j,n,ck] * pose[j,n,(pos,i)]
        wp = ctx.enter_context(tc.tile_pool(name="wp", bufs=2))
        vps_stack = ExitStack()
        vpsA = vps_stack.enter_context(tc.tile_pool(name="vpsA", bufs=2, space="PSUM"))
        vpsB = vps_stack.enter_context(tc.tile_pool(name="vpsB", bufs=2, space="PSUM"))

        for gi, g in enumerate(range(0, NB, 16)):
            if gi in (8, 13):
                hh = 0 if gi == 8 else 1
                nc.sync.dma_start(out=WEh[hh][:],
                                  in_=dram["W0REP"][:, hh * 16:(hh + 1) * 16, :])
                nc.sync.dma_start(out=S16h[hh][:], in_=dram["S16"][hh * 16:(hh + 1) * 16])
            wpt = wp.tile([4, 16, 256], F16, tag="wpt", bufs=3, name=f"wpt_{gi}")
            nc.sync.dma_start(out=wpt[:], in_=dram["WP"][:, g:g + 16, :])
            vpa = vpsA.tile([128, 16, 64], F32, tag="vpa", name=f"vpa_{gi}")
            vpb = vpsB.tile([128, 16, 64], F32, tag="vpb", name=f"vpb_{gi}")
            for t in range(16):
                nc.tensor.matmul(vpa[:, t], lhsT=wpt[:, t, 0:128], rhs=wpt[:, t, 128:192],
                                 start=True, stop=True)
                nc.tensor.matmul(vpb[:, t], lhsT=wpt[:, t, 0:128], rhs=wpt[:, t, 192:256],
                                 start=True, stop=True)
            nc.vector.tensor_copy(
                Vh[0][:, :, :, g:g + 16],
                vpa[:].rearrange("p n (pos i) -> p i pos n", i=4))
            nc.scalar.copy(
                Vh[1][:, :, :, g:g + 16],
                vpb[:].rearrange("p n (pos i) -> p i pos n", i=4))
        vps_stack.close()
        open_em_pools()

        nc.sync.dma_start(out=betau[:], in_=dram["BETAU"][:])
        nc.sync.dma_start(out=betaa[:], in_=dram["BETAA"][:])
        nc.scalar.mul(balam[:], betaa[:], LAM)
        nc.sync.dma_start(out=maskck[:], in_=dram["MASKCK"][:])
        nc.sync.dma_start(out=maskck32[:], in_=dram["MASKCK32"][:])
        nc.sync.dma_start(out=repck[:], in_=dram["REPCK"][:])
        nc.sync.dma_start(out=ecol[:], in_=dram["ECOL"][:])
        nc.sync.dma_start(out=eyer[:], in_=dram["EYER"][:])
        nc.sync.dma_start(out=rsum0t[:], in_=dram["RSUM0"][:])
        # ---- iter-0 mu on PE: mu0[ck, (pos,i)] = sum_(j,n) W * (pose*w0) ----
        mu0p = pools["psB"].tile([128, 128], F32, tag="misc", name="mu0p", space="PSUM")
        for c9 in range(9):
            wf = wp.tile([128, 256], F16, tag="wf", name=f"wf_{c9}")
            nc.sync.dma_start(out=wf[:], in_=dram["WPF"][c9 * 128:(c9 + 1) * 128])
            nc.tensor.matmul(mu0p[:], lhsT=wf[:, 0:128], rhs=wf[:, 128:256],
                             start=(c9 == 0), stop=(c9 == 8))
        for h in range(2):
            mur0 = sb.tile([128, 4, HALF], F32, tag=f"mur{h}", name=f"mur{h}_pe")
            nc.vector.tensor_copy(
                mur0[:], mu0p[:, h * 64:(h + 1) * 64].rearrange(
                    "p (pos i) -> p i pos", i=4))
            st[h]["mu_raw0"] = mur0


        RSTD = 1.0 / (math.sqrt(EPS) + EPS)   # devsum==0 in exact math

        def m_big(h, it):
            V = Vh[h]; s = st[h]
            CF = WEh[h] if it == 0 else Rh[h]
            if it > 0:
                r1 = tsh.tile([128, HALF, 144], F16, tag="t1", bufs=3, name=f"r1_{h}{it}")
                nc.vector.tensor_add(r1[:], Rh[h][:, :, :144], Rh[h][:, :, 144:])
                r2 = tsh.tile([128, HALF, 72], F16, tag="t2", bufs=3, name=f"r2_{h}{it}")
                nc.vector.tensor_add(r2[:], r1[:, :, :72], r1[:, :, 72:])
                r3 = tsh.tile([128, HALF, 36], F16, tag="t3", bufs=2, name=f"r3_{h}{it}")
                nc.vector.tensor_add(r3[:], r2[:, :, :36], r2[:, :, 36:])
                rsum = sb.tile([128, HALF], F32, tag=f"rsum{h}", name=f"rsum{h}_{it}")
                nc.vector.tensor_reduce(rsum[:], r3[:], axis=AX.X, op=OP.add)
                s["rsum"] = rsum
            if it == 0:
                mu_raw = st[h]["mu_raw0"]
            else:
                mu_raw = sb.tile([128, 4, HALF], F32, tag=f"mur{h}", name=f"mur{h}_{it}")
            ev2 = sb.tile([128, 4, HALF], F32, tag=f"ev2{h}", name=f"ev2{h}_{it}")
            for i in range(4):
                U = tsh.tile([128, HALF, NB], F16, tag=f"U{h}", bufs=1, name=f"U{h}_{it}_{i}")
                nc.vector.tensor_mul(U[:], V[:, i], CF[:])
                if it > 0:
                    t1 = tsh.tile([128, HALF, 144], F16, tag="t1", bufs=3, name=f"t1_{h}{it}{i}")
                    nc.vector.tensor_add(t1[:], U[:, :, :144], U[:, :, 144:])
                    t2 = tsh.tile([128, HALF, 72], F16, tag="t2", bufs=3, name=f"t2_{h}{it}{i}")
                    nc.vector.tensor_add(t2[:], t1[:, :, :72], t1[:, :, 72:])
                    t3 = tsh.tile([128, HALF, 36], F16, tag="t3", bufs=2, name=f"t3_{h}{it}{i}")
                    (nc.gpsimd if CFG["t3_pool"] else nc.vector).tensor_add(
                        t3[:], t2[:, :, :36], t2[:, :, 36:])
                    nc.vector.tensor_reduce(mu_raw[:, i], t3[:], axis=AX.X, op=OP.add)
                ta = tsh.tile([128, HALF, 144], F16, tag="t1", bufs=3, name=f"ta_{h}{it}{i}")
                nc.vector.tensor_mul(ta[:], U[:, :, :144], V[:, i, :, :144])
                tb = tsh.tile([128, HALF, 144], F16, tag="t1", bufs=3, name=f"tb_{h}{it}{i}")
                nc.vector.tensor_mul(tb[:], U[:, :, 144:], V[:, i, :, 144:])
                t2a = tsh.tile([128, HALF, 72], F16, tag="t2", bufs=3, name=f"t2a_{h}{it}{i}")
                (nc.gpsimd if CFG["t2a_pool"] else nc.vector).tensor_add(
                    t2a[:], ta[:, :, :72], ta[:, :, 72:])
                t2b = tsh.tile([128, HALF, 72], F16, tag="t2", bufs=3, name=f"t2b_{h}{it}{i}")
                (nc.gpsimd if CFG["t2b_pool"] else nc.vector).tensor_add(
                    t2b[:], tb[:, :, :72], tb[:, :, 72:])
                t2c = tsh.tile([128, HALF, 72], F16, tag="t2", bufs=3, name=f"t2c_{h}{it}{i}")
                (nc.gpsimd if CFG["t2c_pool"] else nc.vector).tensor_add(t2c[:], t2a[:], t2b[:])
                t3s = tsh.tile([128, HALF, 36], F16, tag="t3", bufs=2, name=f"t3s_{h}{it}{i}")
                (nc.gpsimd if CFG["t3s_pool"] else nc.vector).tensor_add(
                    t3s[:], t2c[:, :, :36], t2c[:, :, 36:])
                nc.vector.tensor_reduce(ev2[:, i], t3s[:], axis=AX.X, op=OP.add)
            s["mu_raw"] = mu_raw; s["ev2"] = ev2

        def m_small(h, it):
            s = st[h]
            if it == 0:
                rsum = rsum0t[:, h * 16:(h + 1) * 16]
                mu = s["mu_raw"]; ev2s = s["ev2"]
            else:
                rsum = s["rsum"][:]
                srec = sb.tile([128, HALF], F32, tag=f"srec{h}", name=f"srec{h}_{it}")
                nc.vector.tensor_scalar_add(srec[:], rsum, EPS)
                nc.vector.reciprocal(srec[:], srec[:])
                sr_b = srec[:].rearrange("p (o h) -> p o h", o=1).broadcast_to([128, 4, HALF])
                mu = sb.tile([128, 4, HALF], F32, tag=f"mu{h}", name=f"mu{h}_{it}")
                nc.vector.tensor_mul(mu[:], s["mu_raw"][:], sr_b)
                ev2s = sb.tile([128, 4, HALF], F32, tag=f"ev2s{h}", name=f"ev2s{h}_{it}")
                nc.vector.tensor_mul(ev2s[:], s["ev2"][:], sr_b)
            musq = sb.tile([128, 4, HALF], F32, tag=f"musq{h}", name=f"musq{h}_{it}")
            nc.scalar.square(musq[:], mu[:])
            sig = sb.tile([128, 4, HALF], F32, tag=f"sig{h}", name=f"sig{h}_{it}")
            nc.vector.tensor_sub(sig[:], ev2s[:], musq[:])
            nc.vector.tensor_scalar_add(sig[:], sig[:], EPS)
            nc.vector.tensor_scalar_max(sig[:], sig[:], EPS)
            lns = sb.tile([128, 4, HALF], F32, tag=f"lns{h}", name=f"lns{h}_{it}")
            nc.scalar.activation(lns[:], sig[:], ACT.Ln, scale=1.0)
            nc.scalar.mul(lns[:], lns[:], 0.5)
            bb = betau[:].rearrange("p (i o) -> p i o", o=1).broadcast_to([128, 4, HALF])
            ch = sb.tile([128, 4, HALF], F32, tag=f"ch{h}", name=f"ch{h}_{it}")
            nc.vector.tensor_add(ch[:], lns[:], bb)
            rs_b = rsum.rearrange("p (o h) -> p o h", o=1).broadcast_to([128, 4, HALF])
            nc.vector.tensor_mul(ch[:], ch[:], rs_b)
            chs = sb.tile([128, HALF], F32, tag=f"chs{h}", name=f"chs{h}_{it}")
            nc.vector.tensor_reduce(chs[:], ch[:].rearrange("p i h -> p h i"),
                                    axis=AX.X, op=OP.add)
            cost = pools["psB"].tile([32, HALF], F32, tag="misc", name=f"cost{h}_{it}", space="PSUM")
            nc.tensor.matmul(cost[:], lhsT=maskck32[:], rhs=chs[:], start=True, stop=True)
            costs = sb.tile([32, HALF], F32, tag=f"costs{h}", name=f"costs{h}_{it}")
            nc.scalar.copy(costs[:], cost[:])
            mbs = pools["psB"].tile([32, HALF], F32, tag="misc", name=f"mbs{h}_{it}", space="PSUM")
            nc.tensor.matmul(mbs[:], lhsT=j32[:], rhs=costs[:], start=True, stop=True)
            tt = sb.tile([32, HALF], F32, tag=f"tt{h}", name=f"tt{h}_{it}")
            nc.vector.tensor_sub(tt[:], mbs[:], costs[:])
            aout = sb.tile([32, HALF], F32, tag=f"aout{h}", name=f"aout{h}_{it}")
            nc.scalar.activation(aout[:], tt[:], ACT.Sigmoid,
                                 scale=-LAM * RSTD, bias=balam[:])
            s["mu"] = mu; s["lns"] = lns; s["aout"] = aout; s["rsum_used"] = rsum

        def e_pre(h, it):
            s = st[h]
            rs = sb.tile([128, 4, HALF], F32, tag=f"rs{h}", name=f"rs{h}_{it}")
            nc.scalar.activation(rs[:], s["lns"][:], ACT.Exp, scale=-1.0, bias=nls2[:])
            s["rs"] = rs
            lsum = sb.tile([128, HALF], F32, tag=f"lsum{h}", name=f"lsum{h}_{it}")
            nc.vector.tensor_reduce(lsum[:], s["lns"][:].rearrange("p i h -> p h i"),
                                    axis=AX.X, op=OP.add)
            lck = pools["psB"].tile([32, HALF], F32, tag="misc", name=f"lck{h}_{it}", space="PSUM")
            nc.tensor.matmul(lck[:], lhsT=maskck32[:], rhs=lsum[:], start=True, stop=True)
            la = sb.tile([32, HALF], F32, tag=f"la{h}", name=f"la{h}_{it}")
            nc.vector.tensor_scalar_add(la[:], s["aout"][:], EPS)
            nc.scalar.activation(la[:], la[:], ACT.Ln)
            lnb = sb.tile([32, HALF], F32, tag=f"lnb{h}", name=f"lnb{h}_{it}")
            nc.vector.tensor_sub(lnb[:], la[:], lck[:])
            mx = sb.tile([32, HALF], F32, tag=f"mx{h}", name=f"mx{h}_{it}")
            sh2 = sb.tile([32, HALF], F32, tag=f"sh2{h}", name=f"sh2{h}_{it}")
            nc.vector.tensor_copy(mx[:], lnb[:])
            for w in (16, 8, 4, 2, 1):
                mask = [(i + w) % 32 for i in range(32)]
                nc.vector.stream_shuffle(sh2[:], mx[:], mask)
                nc.vector.tensor_tensor(mx[:], mx[:], sh2[:], op=OP.max)
            lnbp = sb.tile([32, HALF], F32, tag=f"lnbp{h}", name=f"lnbp{h}_{it}")
            nc.vector.tensor_sub(lnbp[:], lnb[:], mx[:])
            nc.vector.tensor_scalar_add(lnbp[:], lnbp[:], SHIFT)
            s["lnbp"] = lnbp

        def e_big(h, it):
            V = Vh[h]; s = st[h]
            mu, rs, lnbp = s["mu"], s["rs"], s["lnbp"]
            E2v = WEh[h][0:32]
            Sp = pools["psB"].tile([16, NB], F32, tag="misc", name=f"sp{h}_{it}", space="PSUM")
            for sub in range(4):
                for i in range(4):
                    E = dep.tile([128, 4, NB], F16, tag="E", name=f"E{h}_{it}_{sub}_{i}")
                    eng = nc.gpsimd if (CFG["eops_pool"] and i == 3) else nc.vector
                    for p4 in range(4):
                        p = sub * 4 + p4
                        eng.tensor_scalar(
                            E[:, p4], V[:, i, p], mu[:, i, p:p + 1], rs[:, i, p:p + 1],
                            op0=OP.subtract, op1=OP.mult)
                    m = CFG["sq_pool"]
                    if m == 2 or (m == 1 and i % 2 == 0):
                        nc.gpsimd.tensor_mul(SQh[h][:, i], E[:], E[:])
                    elif m == 4 or (m == 3 and i % 2 == 0):
                        nc.vector.tensor_mul(SQh[h][:, i], E[:], E[:])
                    else:
                        nc.scalar.square(SQh[h][:, i], E[:])
                for p4 in range(4):
                    p = sub * 4 + p4
                    qa = pools["psA"].tile([32, NB], F32, tag="qa", name=f"qa{h}_{it}_{p}")
                    for i in range(4):
                        nc.tensor.matmul(qa[:], lhsT=maskck[:], rhs=SQh[h][:, i, p4],
                                         start=(i == 0), stop=(i == 3))
                    nc.scalar.activation(E2v[:, p], qa[:], ACT.Exp,
                                         scale=-1.0, bias=lnbp[:, p:p + 1])
                    nc.tensor.matmul(Sp[:], lhsT=ecol[:, p], rhs=E2v[:, p],
                                     start=(p == 0), stop=(p == 15))
            s["Sp"] = Sp

        def e_norm(h, it):
            s = st[h]
            E2v = WEh[h][0:32]
            spp = sb.tile([16, NB], F32, tag=f"spp{h}", name=f"spp{h}_{it}")
            nc.vector.tensor_scalar_add(spp[:], s["Sp"][:], 1e-30)
            nc.vector.reciprocal(spp[:], spp[:])
            g16 = sb.tile([16, NB], F16, tag=f"g{h}", name=f"g{h}_{it}")
            nc.vector.tensor_mul(g16[:], spp[:], S16h[h][:])
            nc.vector.tensor_scalar_min(g16[:], g16[:], 60000.0)
            nc.vector.tensor_scalar_max(g16[:], g16[:], -60000.0)
            for p in range(16):
                Gp = pools["psB"].tile([32, NB], F32, tag="gp", name=f"gp{h}_{it}_{p}", space="PSUM")
                nc.tensor.matmul(Gp[:], lhsT=eyer[:, p, 0:32], rhs=g16[:],
                                 start=True, stop=True)
                if CFG["gmul_pool"]:
                    gs = dep.tile([32, NB], F16, tag="gs", name=f"gs{h}_{it}_{p}")
                    nc.scalar.copy(gs[:], Gp[:])
                    nc.gpsimd.tensor_mul(E2v[:, p], E2v[:, p], gs[:])
                else:
                    nc.vector.tensor_mul(E2v[:, p], E2v[:, p], Gp[:])
            e2f = E2v.rearrange("c h n -> c (h n)")
            rhf = Rh[h][:].rearrange("p h n -> p (h n)")
            for q0 in range(0, NB * HALF, 512):
                E4 = pools["psB"].tile([128, 512], F32, tag="e4", name=f"e4{h}_{it}_{q0}", space="PSUM")
                nc.tensor.matmul(E4[:], lhsT=repck[:], rhs=e2f[:, q0:q0 + 512],
                                 start=True, stop=True)
                nc.scalar.copy(rhf[:, q0:q0 + 512], E4[:])


        # software-pipelined schedule: h1 offset so its DVE-heavy m_big
        # overlaps h0's ACT/PE-heavy e_big/e_norm
        def emit_out(h):
            p0 = h * HALF
            mu = st[h]["mu"]; aout = st[h]["aout"]
            for k in range(4):
                mo = OUT[p0:p0 + HALF, k:512:4].rearrange(
                    "pos (c i) -> c i pos", c=32, i=4)
                nc.sync.dma_start(out=mo, in_=mu[k:128:4])
            ao = OUT[p0:p0 + HALF, 512:544].rearrange("pos c -> c pos")
            nc.sync.dma_start(out=ao, in_=aout[:])

        SCHED = [
            ("mb00", lambda: m_big(0, 0)),
            ("msep00", lambda: (m_small(0, 0), e_pre(0, 0))),
            ("ebig00", lambda: e_big(0, 0)),
            ("mb10", lambda: m_big(1, 0)),
            ("msep10", lambda: (m_small(1, 0), e_pre(1, 0))),
            ("enorm00", lambda: e_norm(0, 0)),
            ("ebig10", lambda: e_big(1, 0)),
            ("mb01", lambda: m_big(0, 1)),
            ("enorm10", lambda: e_norm(1, 0)),
            ("msep01", lambda: (m_small(0, 1), e_pre(0, 1))),
            ("ebig01", lambda: e_big(0, 1)),
            ("mb11", lambda: m_big(1, 1)),
            ("enorm01", lambda: e_norm(0, 1)),
            ("msep11", lambda: (m_small(1, 1), e_pre(1, 1))),
            ("ebig11", lambda: e_big(1, 1)),
            ("mb02", lambda: m_big(0, 2)),
            ("enorm11", lambda: e_norm(1, 1)),
            ("ms02", lambda: m_small(0, 2)),
            ("mb12", lambda: m_big(1, 2)),
            ("out0", lambda: emit_out(0)),
            ("ms12", lambda: m_small(1, 2)),
            ("out1", lambda: emit_out(1)),
        ]
        for nm, fn in SCHED[:CFG.get("nphase", 999)]:
            fn()



    _fix_excess_waits(nc)
    return nc


def _host_inputs(x, beta_u, beta_a, weights):
    b, h, w, cdim = x.shape
    oh = ow = h - KK + 1
    idxs = np.arange(KK)[:, None] + np.arange(oh)[None, :]
    xp = x[:, idxs]
    xp = xp[:, :, :, idxs]
    xp = np.transpose(xp, (0, 1, 3, 2, 4, 5))      # (b, K, K, oh, ow, c)
    p_in = xp[..., :B_CAPS * PSIZE].reshape(b * oh * ow, NB, P, P)
    a_in = xp[..., B_CAPS * PSIZE:].reshape(b * oh * ow, NB)

    WMAT = np.ascontiguousarray(
        weights[0].transpose(2, 0, 1, 3).reshape(4, NB, 128)).astype(np.float16)
    BETAU = beta_u.reshape(32, 4, 4)
    BETAU = np.ascontiguousarray(
        BETAU.transpose(0, 2, 1).reshape(128, 4)).astype(np.float32)
    BETAA = beta_a.reshape(32, 1).astype(np.float32)
    MASKCK = (np.arange(128)[:, None] // 4 == np.arange(32)[None, :])
    REPCK = (np.arange(128)[None, :] // 4 == np.arange(32)[:, None])
    ECOL = np.broadcast_to(np.eye(16, dtype=np.float16)[None], (32, 16, 16))
    EYER = np.broadcast_to(np.eye(16, dtype=np.float16)[:, :, None], (16, 16, 128))

    in_maps = []
    for ci in range(N_CORES):
        sl = slice(ci * NPOS_CORE, (ci + 1) * NPOS_CORE)
        pose_c = p_in[sl]                          # (32, NB, 4, 4) [pos, n, i, j]
        POSE = np.ascontiguousarray(
            pose_c.transpose(3, 1, 0, 2).reshape(4, NB, 128)).astype(np.float16)
        a = a_in[sl].astype(np.float32)            # (32, NB)
        s = a / (a + EPS)
        w0 = s / (s.sum(1, keepdims=True) + 32 * EPS)
        # pose scaled by w0 for the PE-side iter-0 mu: [j, n, pos*4+i] flat
        wfac = np.repeat(w0.T[:, :, None], 4, axis=2).reshape(NB, 128)  # [n,(pos,i)]
        POSEW = (POSE.astype(np.float32) * wfac[None]).astype(np.float16)
        WP = np.ascontiguousarray(np.concatenate([WMAT, POSE], axis=2))
        WPF = np.ascontiguousarray(np.concatenate(
            [WMAT.reshape(4 * NB, 128), POSEW.reshape(4 * NB, 128)], axis=1))
        W0REP = np.ascontiguousarray(np.broadcast_to(
            w0[None], (128, 32, NB))).astype(np.float16)
        RSUM0 = np.ascontiguousarray(np.broadcast_to(
            (s.sum(1) / 32.0)[None], (128, 32))).astype(np.float32)
        in_maps.append(dict(
            WP=WP, WPF=WPF, W0REP=W0REP, S16=s.astype(np.float32),
            RSUM0=RSUM0, BETAU=BETAU, BETAA=BETAA,
            MASKCK=MASKCK.astype(np.float16), MASKCK32=MASKCK.astype(np.float32),
            REPCK=REPCK.astype(np.float16),
            ECOL=np.ascontiguousarray(ECOL), EYER=np.ascontiguousarray(EYER)))
    return in_maps


def kernel(x, beta_u, beta_a, weights):
    x = np.asarray(x, dtype=np.float32)
    beta_u = np.asarray(beta_u, dtype=np.float32)
    beta_a = np.asarray(beta_a, dtype=np.float32)
    weights = np.asarray(weights, dtype=np.float32)

    if "nc" not in _cache:
        _cache["nc"] = _build_nc()
    nc = _cache["nc"]
    in_maps = _host_inputs(x, beta_u, beta_a, weights)
    res = bass_utils.run_bass_kernel_spmd(nc, in_maps, list(range(N_CORES)))
    _cache["last_res"] = res
    outs = [res.results[ci]["OUT"] for ci in range(N_CORES)]
    full = np.concatenate(outs, axis=0)            # (256, 544)
    return full.reshape(4, 8, 8, 544)
